# revision 6
# baseline (speedup 1.0000x reference)
"""Bidirectional 2-layer GRU on 8 Trainium2 NeuronCores (Bass/Tile).

Single SPMD launch; near-straight-line program (one tiny per-chunk If that
selects the gi-input ring fill source).

  core 0: L0-fwd chain    core 1: L1-fwd chain (+ fwd half of fc)
  core 2: L0-bwd chain    core 3: L1-bwd chain (+ bwd half of fc)
  cores 4-7: dummy pair mirrors (same program, inert data)

Pair cores (2d, 2d+1) stream the layer-0 hidden-state transposes h0^T from
the even core to the odd core via per-chunk pair AllGather collectives.
The odd core computes gi1 = h0 @ W_ih_mid^T locally from those tiles and
runs the layer-1 chain; it exports r, z, h1 and an fc partial product.

Backward direction = forward program on host-reversed inputs.

Slot pipeline: NSLOT = T/C + 1. L0 computes chunk k in slot k; the slot-k
AllGather carries the h^T staging written during slot k-1; L1 computes its
chain chunk k-1 in slot k. L1 state is zeroed at slot-1 entry via a
keep-mask multiply (km input: 1.0 on L0 cores, 0.0 on L1 cores).

Export rows: program step t writes gr/gz/ghn row t (= its own chain step)
and fco row t-1 (fc of the previous step's h'); a post-loop fc block fills
the last row, so fco[row] = fc(h'(row)) for every row. L1-core chain step
s lives at program row s + C.
"""

import os
import sys
import time

for _p in ("/opt/trn_rl_repo", "/root/.axon_site/_ro/trn_rl_repo"):
    if os.path.isdir(_p) and _p not in sys.path:
        sys.path.insert(0, _p)

import numpy as np
import ml_dtypes
from contextlib import ExitStack

BF16 = ml_dtypes.bfloat16

B, T, IN, H, OUT = 128, 512, 256, 512, 256
H3 = 3 * H
NCORES = 8

C = int(os.environ.get("GRU_C", "32"))
T_STEPS = int(os.environ.get("GRU_T", str(T)))
NSLOT = T_STEPS // C + 1
TPAD = NSLOT * C


def build_nc():
    import concourse.bass as bass
    import concourse.bacc as bacc
    import concourse.mybir as mybir
    from concourse import tile

    dt = mybir.dt
    AF = mybir.ActivationFunctionType

    nc = bacc.Bacc("TRN2", target_bir_lowering=False, debug=False,
                   num_devices=NCORES)
    tc = tile.TileContext(nc)

    xsrc = nc.dram_tensor("xsrc", [TPAD, 2, 128, B], dt.bfloat16, kind="ExternalInput")
    giw = nc.dram_tensor("giw", [4, 128, H3], dt.bfloat16, kind="ExternalInput")
    ghw = nc.dram_tensor("ghw", [4, 128, H3], dt.bfloat16, kind="ExternalInput")
    fcw = nc.dram_tensor("fcw", [4, 128, OUT], dt.bfloat16, kind="ExternalInput")
    brz = nc.dram_tensor("brz", [128, 2 * H], dt.float32, kind="ExternalInput")
    bin_ = nc.dram_tensor("bin", [128, H], dt.float32, kind="ExternalInput")
    bhn = nc.dram_tensor("bhn", [128, H], dt.float32, kind="ExternalInput")
    ident = nc.dram_tensor("ident", [128, 128], dt.bfloat16, kind="ExternalInput")
    km = nc.dram_tensor("km", [128, 1], dt.float32, kind="ExternalInput")
    role = nc.dram_tensor("role", [1, 1], dt.int32, kind="ExternalInput")

    gr = nc.dram_tensor("gr", [TPAD, 128, H], dt.bfloat16, kind="ExternalOutput")
    gz = nc.dram_tensor("gz", [TPAD, 128, H], dt.bfloat16, kind="ExternalOutput")
    ghn_o = nc.dram_tensor("ghn", [TPAD, 128, H], dt.bfloat16, kind="ExternalOutput")
    fco = nc.dram_tensor("fco", [TPAD, 128, OUT], dt.float32, kind="ExternalOutput")

    RHALF = C * 2 * 128          # cols per ring (A or B) chunk slot
    STG_F = C * 512              # staging cols per chunk (4 tiles x 128)

    with ExitStack() as ctx:
        ctx.enter_context(tc)
        wp = ctx.enter_context(tc.tile_pool(name="wp", bufs=1))
        xp = ctx.enter_context(tc.tile_pool(name="xp", bufs=2))
        sp = ctx.enter_context(tc.tile_pool(name="sp", bufs=2))
        pp = ctx.enter_context(tc.tile_pool(name="pp", bufs=1, space="PSUM"))
        dp = ctx.enter_context(tc.tile_pool(name="dp", bufs=2, space="DRAM"))

        giw_t = wp.tile([128, 4 * H3], dt.bfloat16, tag="giw")
        ghw_t = wp.tile([128, 4 * H3], dt.bfloat16, tag="ghw")
        fcw_t = wp.tile([128, 4 * OUT], dt.bfloat16, tag="fcw")
        for k in range(4):
            nc.sync.dma_start(giw_t[:, k * H3:(k + 1) * H3], giw[k])
            nc.sync.dma_start(ghw_t[:, k * H3:(k + 1) * H3], ghw[k])
            nc.sync.dma_start(fcw_t[:, k * OUT:(k + 1) * OUT], fcw[k])
        brz_t = wp.tile([128, 2 * H], dt.float32, tag="brz")
        nc.sync.dma_start(brz_t[:], brz[:])
        bin_t = wp.tile([128, H], dt.float32, tag="bin")
        nc.sync.dma_start(bin_t[:], bin_[:])
        bhn_t = wp.tile([128, H], dt.float32, tag="bhn")
        nc.sync.dma_start(bhn_t[:], bhn[:])
        id_t = wp.tile([128, 128], dt.bfloat16, tag="ident")
        nc.sync.dma_start(id_t[:], ident[:])
        km_t = wp.tile([128, 1], dt.float32, tag="km")
        nc.sync.dma_start(km_t[:], km[:])

        role_tile = wp.tile([1, 1], dt.int32, tag="role")
        nc.sync.dma_start(role_tile[:], role[0:1, 0:1])
        role_regs = nc.alloc_registers("role_r", mybir.ALL_ENGINES)
        nc.regs_load(role_regs, role_tile[0:1, 0:1])
        role_v = nc.snap(role_regs, donate=True, min_val=0, max_val=1)

        h_t = wp.tile([128, H], dt.float32, tag="hstate")
        nc.vector.memset(h_t[:], 0.0)
        hb_t = wp.tile([128, H], dt.bfloat16, tag="hb")
        nc.vector.memset(hb_t[:], 0.0)
        hT0 = wp.tile([128, H], dt.bfloat16, tag="hT0")
        nc.vector.memset(hT0[:], 0.0)

        ringA = [xp.tile([128, RHALF], dt.bfloat16, tag=f"ringA{j}", name=f"ringA{j}") for j in range(2)]
        ringB = [xp.tile([128, RHALF], dt.bfloat16, tag=f"ringB{j}", name=f"ringB{j}") for j in range(2)]
        for j in range(2):
            nc.vector.memset(ringA[j][:], 0.0)
            nc.vector.memset(ringB[j][:], 0.0)

        psA = [pp.tile([128, 2 * H], dt.float32, tag=f"psA{j}", name=f"psA{j}") for j in range(2)]
        psB = pp.tile([128, H], dt.float32, tag="psB")
        psC = pp.tile([128, H], dt.float32, tag="psC")
        tps = pp.tile([128, H], dt.bfloat16, tag="tps")
        fps = pp.tile([128, OUT], dt.float32, tag="fps")

        stg = [dp.tile([128, STG_F], dt.bfloat16, tag=f"stg{j}", name=f"stg{j}") for j in range(2)]
        bnc = [dp.tile([256, STG_F], dt.bfloat16, tag=f"bnc{j}", name=f"bnc{j}") for j in range(2)]
        zz = wp.tile([128, 512], dt.bfloat16, tag="zz")
        nc.vector.memset(zz[:], 0.0)
        for j in range(2):
            for s in range(C):
                nc.sync.dma_start(stg[j][:, s * 512:(s + 1) * 512], zz[:])

        rep_groups = [[0, 1], [2, 3], [4, 5], [6, 7]]

        state = {"Cb": None, "hT": hT0}

        def gi_input(ring_pair, s, k):
            rA, rB = ring_pair
            if k < 2:
                return rA[:, (s * 2 + k) * 128:(s * 2 + k + 1) * 128]
            return rB[:, (s * 2 + (k - 2)) * 128:(s * 2 + (k - 2) + 1) * 128]

        def emit_gi(t_psA, t_psC, ring_pair, s, start_clear):
            """gi matmuls for step-local index s into (t_psA, t_psC)."""
            for nb in range(2):
                for k in range(4):
                    nc.tensor.matmul(
                        t_psA[:, nb * H:(nb + 1) * H],
                        gi_input(ring_pair, s, k),
                        giw_t[:, k * H3 + nb * H: k * H3 + (nb + 1) * H],
                        start=(start_clear and k == 0), stop=False,
                        skip_group_check=True)
            for k in range(4):
                nc.tensor.matmul(
                    psC[:], gi_input(ring_pair, s, k),
                    giw_t[:, k * H3 + 2 * H: k * H3 + 3 * H],
                    start=(k == 0), stop=(k == 3))
            nc.vector.tensor_add(t_psA[:], t_psA[:], brz_t[:])
            cb = sp.tile([128, H], dt.float32, tag="cb")
            nc.vector.tensor_add(cb[:], psC[:], bin_t[:])
            return cb

        def emit_gh_rz(t_psA, hT):
            for nb in range(2):
                for k in range(4):
                    nc.tensor.matmul(
                        t_psA[:, nb * H:(nb + 1) * H],
                        hT[:, k * 128:(k + 1) * 128],
                        ghw_t[:, k * H3 + nb * H: k * H3 + (nb + 1) * H],
                        start=False, stop=(k == 3),
                        skip_group_check=True)

        def emit_fc(hT, row):
            for k in range(4):
                nc.tensor.matmul(fps[:], hT[:, k * 128:(k + 1) * 128],
                                 fcw_t[:, k * OUT:(k + 1) * OUT],
                                 start=(k == 0), stop=(k == 3))
            fcs = sp.tile([128, OUT], dt.float32, tag="fcs")
            nc.vector.tensor_copy(fcs[:], fps[:])
            nc.sync.dma_start(fco[row], fcs[:])

        def emit_step(t_glob, slot, s, ring_pair, ring_pair_next):
            cur = t_glob % 2
            nxt = 1 - cur
            hT_prev = state["hT"]

            # PE: gh_n(t) -> psB
            for k in range(4):
                nc.tensor.matmul(psB[:], hT_prev[:, k * 128:(k + 1) * 128],
                                 ghw_t[:, k * H3 + 2 * H: k * H3 + 3 * H],
                                 start=(k == 0), stop=(k == 3))

            # ACT/DVE elementwise chain of step t
            rz = sp.tile([128, 2 * H], dt.bfloat16, tag="rz")
            nc.scalar.activation(rz[:], psA[cur][:], AF.Sigmoid)
            gp = sp.tile([128, H], dt.float32, tag="gp")
            nc.vector.tensor_add(gp[:], psB[:], bhn_t[:])
            rh = sp.tile([128, H], dt.float32, tag="rh")
            nc.vector.tensor_mul(rh[:], rz[:, 0:H], gp[:])
            npre = sp.tile([128, H], dt.float32, tag="npre")
            nc.vector.tensor_add(npre[:], rh[:], state["Cb"][:])
            n_t = sp.tile([128, H], dt.float32, tag="nt")
            nc.scalar.activation(n_t[:], npre[:], AF.Tanh)
            d_t = sp.tile([128, H], dt.float32, tag="dt")
            nc.vector.tensor_sub(d_t[:], h_t[:], n_t[:])
            zd = sp.tile([128, H], dt.float32, tag="zd")
            nc.vector.tensor_mul(zd[:], rz[:, H:2 * H], d_t[:])
            nc.vector.tensor_add(h_t[:], n_t[:], zd[:])
            nc.vector.tensor_copy(hb_t[:], h_t[:])

            # PE: fc of previous step (reads hT_prev)
            if t_glob >= 1:
                emit_fc(hT_prev, t_glob - 1)

            # PE: gi(t+1) prefetch (within-chunk only; chunk boundary gi is
            # computed at the next slot's start, after the ring fill)
            t2 = t_glob + 1
            do_prefetch = t2 < NSLOT * C and (s + 1) < C
            if do_prefetch:
                state["Cb"] = emit_gi(psA[nxt], psC, ring_pair, s + 1,
                                      start_clear=True)

            # PE: transposes of h'(t); hT tile for next step
            for k in range(4):
                nc.tensor.transpose(tps[:, k * 128:(k + 1) * 128],
                                    hb_t[:, k * 128:(k + 1) * 128], id_t[:])
            hTn = sp.tile([128, H], dt.bfloat16, tag="hTn")
            nc.vector.tensor_copy(hTn[:], tps[:])
            state["hT"] = hTn

            # PE: gh_rz(t+1) accumulates onto gi+bias in psA[nxt]
            if do_prefetch:
                emit_gh_rz(psA[nxt], hTn)

            # exports
            nc.sync.dma_start(gr[t_glob], rz[:, 0:H])
            nc.sync.dma_start(gz[t_glob], rz[:, H:2 * H])
            nc.sync.dma_start(ghn_o[t_glob], hb_t[:])
            nc.sync.dma_start(stg[slot % 2][:, s * 512:(s + 1) * 512], hTn[:])

        for slot in range(NSLOT):
            rp = (ringA[slot % 2], ringB[slot % 2])
            rp_next = (ringA[(slot + 1) % 2], ringB[(slot + 1) % 2])

            nc.gpsimd.collective_compute(
                "AllGather", mybir.AluOpType.bypass,
                replica_groups=rep_groups,
                ins=[stg[(slot + 1) % 2].opt()],
                outs=[bnc[slot % 2].opt()],
            )

            # ring fill: L0 (role 0) from xsrc; L1 (role 1) from bounce slot 0
            with tc.If(role_v == 0) as cmp:
                xv = xsrc[slot * C:(slot + 1) * C].rearrange("s k p b -> p s k b")
                nc.sync.dma_start(
                    ringA[slot % 2][:, 0:RHALF].rearrange("p (s k b) -> p s k b",
                                                          k=2, b=128), xv)
            with cmp.Else():
                bv = bnc[slot % 2][0:128, :].rearrange("p (s k b) -> p s k b",
                                                       k=4, b=128)
                nc.sync.dma_start(
                    ringA[slot % 2][:, 0:RHALF].rearrange("p (s k b) -> p s k b",
                                                          k=2, b=128),
                    bv[:, :, 0:2, :])
                nc.sync.dma_start(
                    ringB[slot % 2][:, 0:RHALF].rearrange("p (s k b) -> p s k b",
                                                          k=2, b=128),
                    bv[:, :, 2:4, :])

            if slot == 1:
                nc.vector.tensor_scalar_mul(h_t[:], h_t[:], km_t[:, 0:1])
                nc.vector.tensor_scalar_mul(hb_t[:], hb_t[:], km_t[:, 0:1])
                hTm = sp.tile([128, H], dt.bfloat16, tag="hTn")
                nc.vector.tensor_scalar_mul(hTm[:], state["hT"][:], km_t[:, 0:1])
                state["hT"] = hTm

            # slot bootstrap: gi + gh_rz of the slot's first step (reads the
            # freshly filled ring, so no stale-read across the fill DMA)
            first_cur = (slot * C) % 2
            state["Cb"] = emit_gi(psA[first_cur], psC, rp, 0, start_clear=True)
            emit_gh_rz(psA[first_cur], state["hT"])

            for s in range(C):
                emit_step(slot * C + s, slot, s, rp, rp_next)

        # final fc for the last program row
        emit_fc(state["hT"], NSLOT * C - 1)

    nc.compile()
    return nc


# ======================= host side =======================

def _pack_ktiles(w, nk, rows_per=128):
    out = np.zeros((nk, rows_per, w.shape[1]), dtype=BF16)
    r = 0
    for k in range(nk):
        take = min(rows_per, max(0, w.shape[0] - r))
        if take > 0:
            out[k, :take] = w[r:r + take].astype(BF16)
        r += rows_per
    return out


def _rep(v):
    v = np.asarray(v, np.float32)
    return np.broadcast_to(v[None, :], (128, v.shape[0])).copy()


_NC_CACHE = {}
LAST_RAW = None
LAST_EXEC_S = None


def _get_nc():
    key = (T_STEPS, C)
    if key not in _NC_CACHE:
        _NC_CACHE[key] = build_nc()
    return _NC_CACHE[key]


def kernel(inputs, w_ih_in, w_hh_in, b_ih_in, b_hh_in,
           w_ih_mid, w_hh_mid, b_ih_mid, b_hh_mid, fc_w, fc_b):
    inputs = np.asarray(inputs, np.float32)
    w_ih_in = np.asarray(w_ih_in, np.float32)
    w_hh_in = np.asarray(w_hh_in, np.float32)
    w_ih_mid = np.asarray(w_ih_mid, np.float32)
    w_hh_mid = np.asarray(w_hh_mid, np.float32)
    b_ih_in = np.asarray(b_ih_in, np.float32)
    b_hh_in = np.asarray(b_hh_in, np.float32)
    b_ih_mid = np.asarray(b_ih_mid, np.float32)
    b_hh_mid = np.asarray(b_hh_mid, np.float32)
    fc_w = np.asarray(fc_w, np.float32)
    fc_b = np.asarray(fc_b, np.float32)

    nc = _get_nc()
    Tm = T_STEPS

    xfwd = np.zeros((TPAD, 2, 128, B), dtype=BF16)
    xT = inputs[:, :Tm, :].transpose(1, 2, 0)          # [T, IN, B]
    xfwd[:Tm] = xT.reshape(Tm, 2, 128, B).astype(BF16)
    xbwd = np.zeros_like(xfwd)
    xbwd[:Tm] = xfwd[:Tm][::-1]
    xzero = np.zeros_like(xfwd)

    giw_L0 = _pack_ktiles(w_ih_in.T, 4)
    giw_L1 = _pack_ktiles(w_ih_mid.T, 4)
    ghw_L0 = _pack_ktiles(w_hh_in.T, 4)
    ghw_L1 = _pack_ktiles(w_hh_mid.T, 4)
    fcw_f = _pack_ktiles(fc_w[:, 0:H].T, 4)
    fcw_b = _pack_ktiles(fc_w[:, H:2 * H].T, 4)
    fcw_z = np.zeros_like(fcw_f)

    brz_L0 = _rep(b_ih_in[0:2 * H] + b_hh_in[0:2 * H])
    brz_L1 = _rep(b_ih_mid[0:2 * H] + b_hh_mid[0:2 * H])
    bin_L0 = _rep(b_ih_in[2 * H:3 * H])
    bin_L1 = _rep(b_ih_mid[2 * H:3 * H])
    bhn_L0 = _rep(b_hh_in[2 * H:3 * H])
    bhn_L1 = _rep(b_hh_mid[2 * H:3 * H])

    ident = np.eye(128, dtype=BF16)
    km1 = np.ones((128, 1), np.float32)
    km0 = np.zeros((128, 1), np.float32)

    def mk(r, xarr, fcwh):
        return {
            "xsrc": xarr,
            "giw": giw_L0 if r == 0 else giw_L1,
            "ghw": ghw_L0 if r == 0 else ghw_L1,
            "fcw": fcwh,
            "brz": brz_L0 if r == 0 else brz_L1,
            "bin": bin_L0 if r == 0 else bin_L1,
            "bhn": bhn_L0 if r == 0 else bhn_L1,
            "ident": ident,
            "km": km1 if r == 0 else km0,
            "role": np.array([[r]], np.int32),
        }

    in_maps = [
        mk(0, xfwd, fcw_z),
        mk(1, xzero, fcw_f),
        mk(0, xbwd, fcw_z),
        mk(1, xzero, fcw_b),
        mk(0, xzero, fcw_z),
        mk(1, xzero, fcw_z),
        mk(0, xzero, fcw_z),
        mk(1, xzero, fcw_z),
    ]

    fetch = {(1, "gr"), (1, "gz"), (1, "ghn"), (1, "fco"),
             (3, "gr"), (3, "gz"), (3, "ghn"), (3, "fco")}
    if os.environ.get("GRU_DEBUG"):
        fetch |= {(0, "ghn"), (2, "ghn"), (0, "gr"), (0, "gz")}
    res = _run_selective(nc, in_maps, fetch=fetch)
    global LAST_RAW
    LAST_RAW = res

    def rows(core, name):
        return res[core][name][C:C + Tm]

    r_f = rows(1, "gr").astype(np.float32)
    z_f = rows(1, "gz").astype(np.float32)
    h_f = rows(1, "ghn").astype(np.float32)
    p_f = rows(1, "fco")
    r_b = rows(3, "gr").astype(np.float32)[::-1]
    z_b = rows(3, "gz").astype(np.float32)[::-1]
    h_b = rows(3, "ghn").astype(np.float32)[::-1]
    p_b = rows(3, "fco")[::-1]

    outputs = (p_f + p_b + fc_b[None, None, :]).transpose(1, 0, 2)
    gates_r = np.concatenate([r_f, r_b], axis=-1).transpose(1, 0, 2)
    gates_z = np.concatenate([z_f, z_b], axis=-1).transpose(1, 0, 2)
    hn = np.concatenate([h_f, h_b], axis=-1).transpose(1, 0, 2)
    return (np.ascontiguousarray(outputs.astype(np.float32)),
            np.ascontiguousarray(gates_r.astype(np.float32)),
            np.ascontiguousarray(gates_z.astype(np.float32)),
            np.ascontiguousarray(hn.astype(np.float32)))


def _run_selective(nc, in_maps, fetch):
    import jax
    from jax.sharding import Mesh, PartitionSpec
    from jax.experimental.shard_map import shard_map
    import concourse.mybir as mybir
    from concourse.bass2jax import (_bass_exec_p, install_neuronx_cc_hook,
                                    partition_id_tensor)

    install_neuronx_cc_hook()
    n_cores = len(in_maps)
    partition_name = nc.partition_id_tensor.name if nc.partition_id_tensor else None

    in_names, out_names, out_avals, zero_outs = [], [], [], []
    for alloc in nc.m.functions[0].allocations:
        if not isinstance(alloc, mybir.MemoryLocationSet):
            continue
        name = alloc.memorylocations[0].name
        if alloc.kind == "ExternalInput":
            if name != partition_name:
                in_names.append(name)
        elif alloc.kind == "ExternalOutput":
            out_names.append(name)
            shape = tuple(alloc.tensor_shape)
            dtype = mybir.dt.np(alloc.dtype)
            out_avals.append(jax.core.ShapedArray(shape, dtype))
            zero_outs.append(np.zeros(shape, dtype))
    n_params = len(in_names)
    n_outs = len(out_avals)
    in_names = in_names + out_names + ([partition_name] if partition_name else [])

    donate = tuple(range(n_params, n_params + n_outs))

    def _body(*args):
        operands = list(args)
        if partition_name is not None:
            operands.append(partition_id_tensor())
        outs = _bass_exec_p.bind(
            *operands,
            out_avals=tuple(out_avals),
            in_names=tuple(in_names),
            out_names=tuple(out_names),
            lowering_input_output_aliases=(),
            sim_require_finite=True,
            sim_require_nnan=True,
            nc=nc,
        )
        return tuple(outs)

    devices = jax.devices()[:n_cores]
    mesh = Mesh(np.asarray(devices), ("core",))
    in_specs = (PartitionSpec("core"),) * (n_params + n_outs)
    out_specs = (PartitionSpec("core"),) * len(out_names)
    sharded = jax.jit(
        shard_map(_body, mesh=mesh, in_specs=in_specs, out_specs=out_specs,
                  check_rep=False),
        donate_argnums=donate, keep_unused=True)
    per_core = [[np.asarray(m[name]) for name in in_names[:n_params]]
                for m in in_maps]
    concat_in = [np.concatenate([per_core[c][i] for c in range(n_cores)], axis=0)
                 for i in range(n_params)]
    concat_zeros = [np.zeros((n_cores * z.shape[0], *z.shape[1:]), z.dtype)
                    for z in zero_outs]
    import jax as _jax
    t_up0 = time.time()
    # pre-place inputs on devices so the execute timing excludes host upload
    in_shardings = [s_ for s_ in sharded.lower(*concat_in, *concat_zeros)
                    .compile().input_shardings[0]] if False else None
    t_exec0 = time.time()
    out_arrs = sharded(*concat_in, *concat_zeros)
    for oa in out_arrs:
        oa.block_until_ready()
    t_exec1 = time.time()
    global LAST_EXEC_S
    LAST_EXEC_S = t_exec1 - t_exec0

    result = {}
    for i, name in enumerate(out_names):
        cores_wanted = {c for (c, n) in fetch if n == name}
        if not cores_wanted:
            continue
        per_core_rows = out_avals[i].shape[0]
        for shard in out_arrs[i].addressable_shards:
            start = shard.index[0].start or 0
            c = start // per_core_rows
            if c in cores_wanted:
                result.setdefault(c, {})[name] = np.asarray(shard.data)
    return result


if __name__ == "__main__":
    t0 = time.time()
    nc = build_nc()
    print(f"build+compile ok in {time.time()-t0:.1f}s")
